# revision 1
# baseline (speedup 1.0000x reference)
"""BSI quantized linear kernel for Trainium2 (8 NeuronCores, SPMD).

Computes out = round(x*100)/100 @ (round(W*100)/100).T + b for
x [4096, 4096] f32, W [4096, 4096] f32, b [4096] f32.

Sharding: W and b are sharded over out_features across the 8 cores
(tensor/column parallel); x is replicated. Each core computes its
[4096, 512] slice of the output; the host concatenates slices.

Math strategy: the quantized values round(100*v) are small integers
(|.| <= ~550 for x, <= ~11 for W), exactly representable in fp16.
The GEMM runs in fp16 on the PE at full rate, accumulating exact
integer dot products in fp32 PSUM (|sum| << 2^24), then the result is
scaled by 1e-4 and bias is added. Rounding uses the fp32 magic-number
trick (+/- 1.5*2^23) which implements round-half-to-even, matching
jnp.round bit-for-bit on the integer grid.

Per-core pipeline (B-row stripes of 128):
  DMA x stripe (f32, natural layout)
  ACT:  t = 100*x + MAGIC            (f32, in place)
  POOL: q = t - MAGIC -> fp16        (integer-valued fp16)
  PE:   transpose 128x128 q blocks -> PSUM (d on partitions)
  DVE:  copy PSUM -> SBUF xT tiles
  PE:   32-step K accumulation matmul vs resident quantized W^T
  ACT:  out_sbuf = 1e-4 * psum
  DVE:  out_sbuf += bias (broadcast)
  DMA out stripe
"""

import numpy as np

_B, _D, _DOUT = 4096, 4096, 4096
_NCORES = 8
_OPER = _DOUT // _NCORES  # 512
_MAGIC = 12582912.0  # 1.5 * 2**23
_P = 128

_nc_cache = {}


def _build(B, D, OPER):
    import concourse.mybir as mybir
    import concourse.tile as tile
    from concourse import bacc
    from concourse.masks import make_identity

    f32 = mybir.dt.float32
    f16 = mybir.dt.float16
    Copy = mybir.ActivationFunctionType.Copy
    P = _P
    KT = D // P
    BT = B // P
    OT = OPER // P
    KG = 8  # transposed 128x128 fp16 blocks per PSUM bank
    NG = KT // KG

    nc = bacc.Bacc("TRN2", target_bir_lowering=False, debug=False,
                   num_devices=_NCORES)
    x_d = nc.dram_tensor("x", [B, D], f32, kind="ExternalInput").ap()
    w_d = nc.dram_tensor("w", [OPER, D], f32, kind="ExternalInput").ap()
    b_d = nc.dram_tensor("b", [OPER], f32, kind="ExternalInput").ap()
    o_d = nc.dram_tensor("out", [B, OPER], f32, kind="ExternalOutput").ap()

    with tile.TileContext(nc) as tc:
        with (
            tc.tile_pool(name="const", bufs=1) as cpool,
            tc.tile_pool(name="wq", bufs=1) as wpool,
            tc.tile_pool(name="stage", bufs=3) as spool,
            tc.tile_pool(name="q16", bufs=3) as qpool,
            tc.tile_pool(name="xT", bufs=3) as xtpool,
            tc.tile_pool(name="tps", bufs=3, space="PSUM") as tppool,
            tc.tile_pool(name="mmps", bufs=2, space="PSUM") as mmpool,
            tc.tile_pool(name="osb", bufs=3) as opool,
            tc.tile_pool(name="wstg", bufs=1) as wstgpool,
        ):
            ident = cpool.tile([P, P], f16)
            make_identity(nc, ident)
            bias_bc = cpool.tile([P, OPER], f32)
            nc.sync.dma_start(bias_bc, b_d[None, :].to_broadcast((P, OPER)))

            # Quantized, transposed W slice, SBUF-resident: [128, KT, OPER] fp16
            wT = wpool.tile([P, KT, OPER], f16)

            def load_quant_transpose(src_rows, dst_cols_fn):
                """DMA 128 rows x D f32, quantize to integer fp16, PE-transpose
                all KT 128x128 blocks, landing them via dst_cols_fn(g) slices."""
                st = spool.tile([P, D], f32, tag="stage")
                nc.sync.dma_start(st, src_rows)
                # t = fl32(fl32(100*x) + MAGIC): the DVE two-stage ALU rounds
                # to f32 between stages, so stage0 reproduces the reference's
                # f32 multiply and stage1's +1.5*2^23 rounds half-to-even to
                # the integer grid.
                nc.vector.tensor_scalar(st, st, 100.0, _MAGIC,
                                        mybir.AluOpType.mult,
                                        mybir.AluOpType.add)
                q = qpool.tile([P, D], f16, tag="q16")
                # subtract the magic constant back out (exact FMA, bias only)
                nc.scalar.activation(q, st, Copy, bias=-_MAGIC, scale=1.0)
                for g in range(NG):
                    tp = tppool.tile([P, KG, P], f16, tag="tps")
                    for j in range(KG):
                        kt = g * KG + j
                        nc.tensor.transpose(tp[:, j, :],
                                            q[:, kt * P:(kt + 1) * P], ident)
                    nc.vector.tensor_copy(dst_cols_fn(g), tp)

            # W preamble: quantize + PE-transpose the W slice into a staging
            # tile, then publish it to wT with a single copy. The GEMM's 1024
            # matmuls then depend on exactly one producer instruction —
            # multi-writer wT was measured to poison the whole matmul stream.
            wstage = wstgpool.tile([P, KT, OPER], f16)
            for ot in range(OT):
                load_quant_transpose(
                    w_d[ot * P:(ot + 1) * P, :],
                    lambda g, ot=ot: wstage[:, g * KG:(g + 1) * KG,
                                            ot * P:(ot + 1) * P],
                )
            nc.vector.tensor_copy(wT, wstage)

            # Main loop over B stripes
            for bt in range(BT):
                xT = xtpool.tile([P, KT, P], f16, tag="xT")
                load_quant_transpose(
                    x_d[bt * P:(bt + 1) * P, :],
                    lambda g, xT=xT: xT[:, g * KG:(g + 1) * KG, :],
                )
                ps = mmpool.tile([P, OPER], f32, tag="mmps")
                for kt in range(KT):
                    nc.tensor.matmul(ps, xT[:, kt, :], wT[:, kt, :],
                                     start=(kt == 0), stop=(kt == KT - 1))
                ob = opool.tile([P, OPER], f32, tag="osb")
                nc.scalar.activation(ob, ps, Copy, bias=0.0, scale=1e-4)
                nc.vector.tensor_add(ob, ob, bias_bc)
                nc.sync.dma_start(o_d[bt * P:(bt + 1) * P, :], ob)

    nc.compile()
    return nc


def _get_nc(B=_B, D=_D, OPER=_OPER):
    key = (B, D, OPER)
    if key not in _nc_cache:
        _nc_cache[key] = _build(B, D, OPER)
    return _nc_cache[key]


def _run(x, W, b, trace=False):
    from concourse.bass_utils import run_bass_kernel_spmd

    B, D = x.shape
    OALL = W.shape[0]
    OPER = OALL // _NCORES
    nc = _get_nc(B, D, OPER)
    in_maps = []
    for c in range(_NCORES):
        in_maps.append({
            "x": x,
            "w": np.ascontiguousarray(W[c * OPER:(c + 1) * OPER]),
            "b": np.ascontiguousarray(b[c * OPER:(c + 1) * OPER]),
        })
    res = run_bass_kernel_spmd(nc, in_maps, core_ids=list(range(_NCORES)),
                               trace=trace)
    out = np.concatenate([res.results[c]["out"] for c in range(_NCORES)],
                         axis=1)
    return out, res


def kernel(x=None, W=None, b=None):
    x = np.ascontiguousarray(np.asarray(x, dtype=np.float32))
    W = np.ascontiguousarray(np.asarray(W, dtype=np.float32))
    b = np.ascontiguousarray(np.asarray(b, dtype=np.float32))
    out, _ = _run(x, W, b, trace=False)
    return out


# --- helpers used by test.py (1x8 column-parallel layout) ---
_R = 1
_C = _NCORES


def _make_in_maps(x, W, b):
    OPER = W.shape[0] // _NCORES
    in_maps = []
    for c in range(_NCORES):
        in_maps.append({
            "x": x,
            "w": np.ascontiguousarray(W[c * OPER:(c + 1) * OPER]),
            "b": np.ascontiguousarray(b[c * OPER:(c + 1) * OPER]),
        })
    return in_maps


def _assemble(outs):
    return np.concatenate(outs, axis=1)



# revision 4
# speedup vs baseline: 1.0628x; 1.0628x over previous
"""BSI quantized linear kernel for Trainium2 (8 NeuronCores, SPMD).

Computes out = round(x*100)/100 @ (round(W*100)/100).T + b for
x [4096, 4096] f32, W [4096, 4096] f32, b [4096] f32.

Sharding: 2x4 grid. x is sharded over the token dim into 2 halves
(rows 0-2047 / 2048-4095); W and b are sharded over out_features into
4 quarters. Core (r, c) = id r*4+c computes out[r*2048:(r+1)*2048,
c*1024:(c+1)*1024]; the host assembles the 8 blocks.

Inputs are handed to each core pre-transposed (x^T and W^T slices,
host-side numpy layout change only — all values bit-identical), so
both matmul operands arrive with the contraction dim on partitions and
the PE does no transposes at all: its only work is the 1024 chained
128x512 fp16 matmuls (~221 us at full rate).

Math strategy (unchanged from the exact baseline): round(100*v) values
are small integers (|.| <= ~550 for x, <= ~13 for W), exactly
representable in fp16. DVE computes fl32(100*v) + 1.5*2^23 (the f32
magic constant forces round-half-to-even to the integer grid, matching
jnp.round bit-for-bit), ACT subtracts the magic back out and emits
fp16. The GEMM accumulates exact integer dots in f32 PSUM; the
epilogue scales by 1e-4 and adds bias.

Per-core schedule:
  preamble: W^T quarter (16 MiB) DMA+quantize, interleaved kt-by-kt
    with x band 0 so the first matmul chain starts after ~1 MiB of DMA
    instead of after the whole W preamble.
  4 bands of 512 token-columns: band 0 issues matmuls kt-outer across
    all 8 PSUM banks (rides the DMA ramp); bands 1-3 are fully
    prefetched and run chain-sequential (staggers epilogues).
"""

import numpy as np

_B, _D, _DOUT = 4096, 4096, 4096
_NCORES = 8
_R, _C = 2, 4              # grid: 2 token-halves x 4 out_feature-quarters
_BPER = _B // _R           # 2048 tokens per core
_OPER = _DOUT // _C        # 1024 out features per core
_MAGIC = 12582912.0        # 1.5 * 2**23
_P = 128

_nc_cache = {}


def _build(BPER, D, OPER):
    import concourse.mybir as mybir
    import concourse.tile as tile
    from concourse import bacc

    f32 = mybir.dt.float32
    f16 = mybir.dt.float16
    Copy = mybir.ActivationFunctionType.Copy
    P = _P
    KT = D // P            # 32 contraction tiles
    NB = 4                 # x bands
    BBAND = BPER // NB     # 512 tokens per band
    SBT = BBAND // P       # 4 stripes per band
    NH = OPER // 512       # 2 o-halves (matmul moving dim is 512 max)

    nc = bacc.Bacc("TRN2", target_bir_lowering=False, debug=False,
                   num_devices=_NCORES)
    xt_d = nc.dram_tensor("xt", [D, BPER], f32, kind="ExternalInput").ap()
    wt_d = nc.dram_tensor("wt", [D, OPER], f32, kind="ExternalInput").ap()
    b_d = nc.dram_tensor("b", [OPER], f32, kind="ExternalInput").ap()
    o_d = nc.dram_tensor("out", [BPER, OPER], f32, kind="ExternalOutput").ap()

    with tile.TileContext(nc) as tc:
        with (
            tc.tile_pool(name="const", bufs=1) as cpool,
            tc.tile_pool(name="wq", bufs=1) as wqpool,
            tc.tile_pool(name="xq", bufs=2) as xqpool,
            tc.tile_pool(name="wstg", bufs=2) as wstgpool,
            tc.tile_pool(name="xstg", bufs=3) as xstgpool,
            tc.tile_pool(name="mm", bufs=8, space="PSUM") as mmpool,
            tc.tile_pool(name="osb", bufs=4) as opool,
        ):
            bias_bc = cpool.tile([P, OPER], f32)
            nc.sync.dma_start(bias_bc, b_d[None, :].to_broadcast((P, OPER)))

            # Resident quantized W^T: 32 single-writer tiles [128, 1024] f16
            wq = [wqpool.tile([P, OPER], f16, tag=f"wq{kt}",
                              name=f"wq{kt}")
                  for kt in range(KT)]

            def quant(dst16, src32):
                # fl32(fl32(100*v) + MAGIC) on DVE (round-half-to-even to
                # the integer grid), then ACT subtracts MAGIC, emits fp16.
                nc.vector.tensor_scalar(src32, src32, 100.0, _MAGIC,
                                        mybir.AluOpType.mult,
                                        mybir.AluOpType.add)
                nc.scalar.activation(dst16, src32, Copy, bias=-_MAGIC,
                                     scale=1.0)

            def load_w_kt(kt):
                st = wstgpool.tile([P, OPER], f32, tag="wst")
                nc.sync.dma_start(st, wt_d[kt * P:(kt + 1) * P, :])
                quant(wq[kt], st)

            def load_x_kt(band, kt):
                st = xstgpool.tile([P, BBAND], f32, tag="xst")
                nc.sync.dma_start(
                    st, xt_d[kt * P:(kt + 1) * P,
                             band * BBAND:(band + 1) * BBAND])
                t = xqpool.tile([P, BBAND], f16, tag=f"xq{kt}")
                quant(t, st)
                return t

            def load_band(band):
                return [load_x_kt(band, kt) for kt in range(KT)]

            chains = [(bt, oh) for bt in range(SBT) for oh in range(NH)]

            def compute_band(xqt, kt_outer):
                ps = [mmpool.tile([P, 512], f32, tag="ps", name=f"ps{j}")
                      for j in range(len(chains))]
                if kt_outer:
                    for kt in range(KT):
                        for j, (bt, oh) in enumerate(chains):
                            nc.tensor.matmul(
                                ps[j], xqt[kt][:, bt * P:(bt + 1) * P],
                                wq[kt][:, oh * 512:(oh + 1) * 512],
                                start=(kt == 0), stop=(kt == KT - 1))
                else:
                    for j, (bt, oh) in enumerate(chains):
                        for kt in range(KT):
                            nc.tensor.matmul(
                                ps[j], xqt[kt][:, bt * P:(bt + 1) * P],
                                wq[kt][:, oh * 512:(oh + 1) * 512],
                                start=(kt == 0), stop=(kt == KT - 1))
                return ps

            def epilogue(band, ps):
                for j, (bt, oh) in enumerate(chains):
                    ob = opool.tile([P, 512], f32, tag="ob")
                    nc.scalar.activation(ob, ps[j], Copy, bias=0.0,
                                         scale=1e-4)
                    nc.vector.tensor_add(
                        ob, ob, bias_bc[:, oh * 512:(oh + 1) * 512])
                    row = (band * SBT + bt) * P
                    nc.sync.dma_start(
                        o_d[row:row + P, oh * 512:(oh + 1) * 512], ob)

            # Ramp: interleave W-kt and band0-kt loads so the first chain
            # can start after one (wq[0], xq[0]) pair instead of the whole
            # 16 MiB W preamble.
            xq_tiles = [None] * NB
            xq_tiles[0] = []
            for kt in range(KT):
                load_w_kt(kt)
                xq_tiles[0].append(load_x_kt(0, kt))
            xq_tiles[1] = load_band(1)

            for band in range(NB):
                ps = compute_band(xq_tiles[band], kt_outer=(band == 0))
                if band + 2 < NB:
                    xq_tiles[band + 2] = load_band(band + 2)
                epilogue(band, ps)

    nc.compile()
    return nc


def _get_nc(BPER=_BPER, D=_D, OPER=_OPER):
    key = (BPER, D, OPER)
    if key not in _nc_cache:
        _nc_cache[key] = _build(BPER, D, OPER)
    return _nc_cache[key]


def _make_in_maps(x, W, b):
    xt = [np.ascontiguousarray(x[r * _BPER:(r + 1) * _BPER, :].T)
          for r in range(_R)]
    wt = [np.ascontiguousarray(W[c * _OPER:(c + 1) * _OPER, :].T)
          for c in range(_C)]
    bs = [np.ascontiguousarray(b[c * _OPER:(c + 1) * _OPER])
          for c in range(_C)]
    in_maps = []
    for r in range(_R):
        for c in range(_C):
            in_maps.append({"xt": xt[r], "wt": wt[c], "b": bs[c]})
    return in_maps


def _assemble(blocks):
    return np.block([[blocks[r * _C + c] for c in range(_C)]
                     for r in range(_R)])


def _run(x, W, b, trace=False):
    from concourse.bass_utils import run_bass_kernel_spmd

    nc = _get_nc()
    in_maps = _make_in_maps(x, W, b)
    res = run_bass_kernel_spmd(nc, in_maps, core_ids=list(range(_NCORES)),
                               trace=trace)
    out = _assemble([res.results[c]["out"] for c in range(_NCORES)])
    return out, res


def kernel(x=None, W=None, b=None):
    x = np.ascontiguousarray(np.asarray(x, dtype=np.float32))
    W = np.ascontiguousarray(np.asarray(W, dtype=np.float32))
    b = np.ascontiguousarray(np.asarray(b, dtype=np.float32))
    out, _ = _run(x, W, b, trace=False)
    return out


# revision 6
# speedup vs baseline: 1.1138x; 1.0480x over previous
"""BSI quantized linear kernel for Trainium2 (8 NeuronCores, SPMD).

Computes out = round(x*100)/100 @ (round(W*100)/100).T + b for
x [4096, 4096] f32, W [4096, 4096] f32, b [4096] f32.

Sharding: 2x4 grid. x is sharded over the token dim into 2 halves;
W and b over out_features into 4 quarters. Core (r, c) = id r*4+c
computes out[r*2048:(r+1)*2048, c*1024:(c+1)*1024]; the host
assembles the 8 blocks.

Inputs are handed to each core pre-transposed (x^T and W^T slices —
host-side numpy layout change only, values bit-identical), so both
matmul operands arrive with the contraction dim on partitions and the
PE does no transposes: its only work is 1024 chained 128x512 fp16
matmuls (~221 us at full rate, measured 216 ns issue-to-issue).

Quantization (exact): DVE computes fl32(100*v) + 1.5*2^23 (f32 magic
constant => round-half-to-even onto the integer grid, matching
jnp.round bitwise), ACT subtracts the magic and emits fp16 (integers
|.|<=~550 are exact in fp16). PSUM accumulates exact integer dots in
f32; epilogue computes 1e-4*psum + bias in one DVE op.

DMA plan (the v2 kernel was trigger-bound: ~200 small DMAs serialized
on the Sync queue at ~230 GB/s effective):
  - W loads as 8 fat chunks of 4 k-tiles ([512,1024] f32 -> [128,4,
    1024], 2 MiB each), interleaved with x band-0 chunks on the Sync
    HWDGE queue so the first matmul chain starts after ~1 chunk.
  - x loads as 8-k-tile chunks ([1024,512] f32 -> [128,8,512], 2 MiB)
    per 512-token band, 4 bands, double-buffered.
  - Output DMAs ([128,1024] per b-stripe) issue from the Scalar
    (Activation) HWDGE queue so they never head-of-line block input
    DMA triggers on the Sync queue.
Band 0 issues matmuls kt-outer across all 8 PSUM banks (rides the DMA
ramp); bands 1-3 are prefetched and run chain-sequential with per-
stripe epilogues.
"""

import numpy as np

_B, _D, _DOUT = 4096, 4096, 4096
_NCORES = 8
_R, _C = 2, 4              # grid: 2 token-halves x 4 out_feature-quarters
_BPER = _B // _R           # 2048 tokens per core
_OPER = _DOUT // _C        # 1024 out features per core
_MAGIC = 12582912.0        # 1.5 * 2**23
_P = 128

_nc_cache = {}


def _build(BPER, D, OPER):
    import concourse.mybir as mybir
    import concourse.tile as tile
    from concourse import bacc

    f32 = mybir.dt.float32
    f16 = mybir.dt.float16
    P = _P
    KT = D // P            # 32 contraction k-tiles
    WCK = 4                # k-tiles per W chunk
    NWC = KT // WCK        # 8 W chunks
    XCK = 8                # k-tiles per x chunk
    NXC = KT // XCK        # 4 x chunks per band
    NB = 4                 # x bands
    BBAND = BPER // NB     # 512 tokens per band
    SBT = BBAND // P       # 4 stripes per band
    NH = OPER // 512       # 2 o-halves (moving dim is 512 max)
    mult = mybir.AluOpType.mult
    add = mybir.AluOpType.add

    nc = bacc.Bacc("TRN2", target_bir_lowering=False, debug=False,
                   num_devices=_NCORES)
    xt_d = nc.dram_tensor("xt", [D, BPER], f32, kind="ExternalInput").ap()
    wt_d = nc.dram_tensor("wt", [D, OPER], f32, kind="ExternalInput").ap()
    b_d = nc.dram_tensor("b", [OPER], f32, kind="ExternalInput").ap()
    o_d = nc.dram_tensor("out", [BPER, OPER], f32, kind="ExternalOutput").ap()

    with tile.TileContext(nc) as tc:
        with (
            tc.tile_pool(name="const", bufs=1) as cpool,
            tc.tile_pool(name="wq", bufs=1) as wqpool,
            tc.tile_pool(name="xq", bufs=2) as xqpool,
            tc.tile_pool(name="wstg", bufs=2) as wstgpool,
            tc.tile_pool(name="xstg", bufs=2) as xstgpool,
            tc.tile_pool(name="mm", bufs=8, space="PSUM") as mmpool,
            tc.tile_pool(name="osb", bufs=2) as opool,
        ):
            bias_bc = cpool.tile([P, OPER], f32)
            nc.sync.dma_start(bias_bc, b_d[None, :].to_broadcast((P, OPER)))

            wq = [wqpool.tile([P, WCK, OPER], f16, tag=f"wq{c}",
                              name=f"wq{c}")
                  for c in range(NWC)]

            def quant(dst16, src32):
                # fl32(fl32(100*v) + MAGIC) on DVE (rounds half-to-even
                # onto the integer grid), ACT subtracts MAGIC -> fp16.
                nc.vector.tensor_scalar(src32, src32, 100.0, _MAGIC,
                                        mult, add)
                nc.scalar.activation(dst16, src32,
                                     mybir.ActivationFunctionType.Copy,
                                     bias=-_MAGIC, scale=1.0)

            def load_w_chunk(c):
                st = wstgpool.tile([P, WCK, OPER], f32, tag="wst",
                                   name=f"wst{c}")
                src = wt_d[c * WCK * P:(c + 1) * WCK * P, :].rearrange(
                    "(k p) o -> p k o", p=P)
                nc.sync.dma_start(st, src)
                quant(wq[c], st)

            def load_x_chunk(band, c):
                st = xstgpool.tile([P, XCK, BBAND], f32, tag="xst",
                                   name=f"xst{band}_{c}")
                src = xt_d[c * XCK * P:(c + 1) * XCK * P,
                           band * BBAND:(band + 1) * BBAND].rearrange(
                    "(k p) b -> p k b", p=P)
                nc.sync.dma_start(st, src)
                t = xqpool.tile([P, XCK, BBAND], f16, tag=f"xq{c}",
                                name=f"xq{band}_{c}")
                quant(t, st)
                return t

            def load_band(band):
                return [load_x_chunk(band, c) for c in range(NXC)]

            def mm(ps, xqt, kt, bt, oh, start, stop):
                nc.tensor.matmul(
                    ps,
                    xqt[kt // XCK][:, kt % XCK, bt * P:(bt + 1) * P],
                    wq[kt // WCK][:, kt % WCK, oh * 512:(oh + 1) * 512],
                    start=start, stop=stop)

            def epilogue_bt(band, bt, ps_pair):
                ob = opool.tile([P, OPER], f32, tag="ob",
                                name=f"ob{band}_{bt}")
                for oh in range(NH):
                    nc.vector.scalar_tensor_tensor(
                        ob[:, oh * 512:(oh + 1) * 512], ps_pair[oh], 1e-4,
                        bias_bc[:, oh * 512:(oh + 1) * 512], mult, add)
                row = (band * SBT + bt) * P
                nc.scalar.dma_start(o_d[row:row + P, :], ob)

            # Ramp: interleave W chunks with band-0 x chunks on the Sync
            # queue so matmuls start after ~1 chunk-pair, then prefetch
            # band 1.
            xq_tiles = [None] * NB
            xq_tiles[0] = []
            for c in range(NWC):
                load_w_chunk(c)
                if c < NXC:
                    xq_tiles[0].append(load_x_chunk(0, c))
            xq_tiles[1] = load_band(1)

            chains = [(bt, oh) for bt in range(SBT) for oh in range(NH)]

            # Band 0: kt-outer across all 8 PSUM banks (DMA-paced ramp).
            ps0 = [mmpool.tile([P, 512], f32, tag="ps", name=f"ps0_{j}")
                   for j in range(len(chains))]
            for kt in range(KT):
                for j, (bt, oh) in enumerate(chains):
                    mm(ps0[j], xq_tiles[0], kt, bt, oh,
                       start=(kt == 0), stop=(kt == KT - 1))
            xq_tiles[2] = load_band(2)
            for bt in range(SBT):
                epilogue_bt(0, bt, ps0[bt * NH:(bt + 1) * NH])

            # Bands 1-3: fully prefetched, chain-sequential with
            # per-stripe epilogues.
            for band in range(1, NB):
                for bt in range(SBT):
                    ps_pair = []
                    for oh in range(NH):
                        ps = mmpool.tile([P, 512], f32, tag="ps",
                                         name=f"ps{band}_{bt}_{oh}")
                        for kt in range(KT):
                            mm(ps, xq_tiles[band], kt, bt, oh,
                               start=(kt == 0), stop=(kt == KT - 1))
                        ps_pair.append(ps)
                    epilogue_bt(band, bt, ps_pair)
                    if band + 2 < NB and bt == 0:
                        xq_tiles[band + 2] = load_band(band + 2)

    nc.compile()
    return nc


def _get_nc(BPER=_BPER, D=_D, OPER=_OPER):
    key = (BPER, D, OPER)
    if key not in _nc_cache:
        _nc_cache[key] = _build(BPER, D, OPER)
    return _nc_cache[key]


def _make_in_maps(x, W, b):
    xt = [np.ascontiguousarray(x[r * _BPER:(r + 1) * _BPER, :].T)
          for r in range(_R)]
    wt = [np.ascontiguousarray(W[c * _OPER:(c + 1) * _OPER, :].T)
          for c in range(_C)]
    bs = [np.ascontiguousarray(b[c * _OPER:(c + 1) * _OPER])
          for c in range(_C)]
    in_maps = []
    for r in range(_R):
        for c in range(_C):
            in_maps.append({"xt": xt[r], "wt": wt[c], "b": bs[c]})
    return in_maps


def _assemble(blocks):
    return np.block([[blocks[r * _C + c] for c in range(_C)]
                     for r in range(_R)])


def _run(x, W, b, trace=False):
    from concourse.bass_utils import run_bass_kernel_spmd

    nc = _get_nc()
    in_maps = _make_in_maps(x, W, b)
    res = run_bass_kernel_spmd(nc, in_maps, core_ids=list(range(_NCORES)),
                               trace=trace)
    out = _assemble([res.results[c]["out"] for c in range(_NCORES)])
    return out, res


def kernel(x=None, W=None, b=None):
    x = np.ascontiguousarray(np.asarray(x, dtype=np.float32))
    W = np.ascontiguousarray(np.asarray(W, dtype=np.float32))
    b = np.ascontiguousarray(np.asarray(b, dtype=np.float32))
    out, _ = _run(x, W, b, trace=False)
    return out


# revision 7
# speedup vs baseline: 1.2202x; 1.0956x over previous
"""BSI quantized linear kernel for Trainium2 (8 NeuronCores, SPMD).

Computes out = round(x*100)/100 @ (round(W*100)/100).T + b for
x [4096, 4096] f32, W [4096, 4096] f32, b [4096] f32.

Sharding: 2x4 grid. x is sharded over the token dim into 2 halves;
W and b over out_features into 4 quarters. Core (r, c) = id r*4+c
computes out[r*2048:(r+1)*2048, c*1024:(c+1)*1024]; the host
assembles the 8 blocks.

Inputs are handed to each core pre-transposed (x^T and W^T slices —
host-side numpy layout change only, values bit-identical), so both
matmul operands arrive with the contraction dim on partitions and the
PE does no transposes: its only work is 1024 chained 128x512 fp16
matmuls (216 ns issue-to-issue measured => ~221 us at full rate).

Quantization (exact): DVE computes fl32(100*v) + 1.5*2^23 (f32 magic
constant => round-half-to-even onto the integer grid, matching
jnp.round bitwise), ACT subtracts the magic and emits fp16 (integers
|.|<=~550 are exact in fp16). PSUM accumulates exact integer dots in
f32; epilogue computes 1e-4*psum + bias in one DVE op per o-half.

Scheduling notes (from traces of earlier revisions):
  - Small per-kt DMAs serialize on HWDGE trigger processing (~230
    GB/s effective): load multi-kt chunks with one trigger each.
  - 3D SBUF tiles cost ~2x on ACT and +15% on matmul operands: all
    compute tiles are 2D; only the DMA *access patterns* are 3D
    (dst tile viewed p (k o) -> p k o, src DRAM (k p) o -> p k o).
  - Output DMAs issue from the Scalar (Activation) HWDGE queue so
    they never head-of-line block input triggers on the Sync queue.
  - W chunks (2 k-tiles) interleave with x band-0 chunks (4 k-tiles)
    so the first matmul chain starts after ~2 MiB of DMA; band 0
    issues matmuls kt-outer across all 8 PSUM banks to ride the DMA
    ramp; bands 1-3 are prefetched and run chain-sequential.
  - Band-0 epilogues issue before the band-2 prefetch so the PSUM
    banks recycle promptly on DVE.
"""

import numpy as np

_B, _D, _DOUT = 4096, 4096, 4096
_NCORES = 8
_R, _C = 2, 4              # grid: 2 token-halves x 4 out_feature-quarters
_BPER = _B // _R           # 2048 tokens per core
_OPER = _DOUT // _C        # 1024 out features per core
_MAGIC = 12582912.0        # 1.5 * 2**23
_P = 128

_nc_cache = {}


def _build(BPER, D, OPER):
    import concourse.mybir as mybir
    import concourse.tile as tile
    from concourse import bacc

    f32 = mybir.dt.float32
    f16 = mybir.dt.float16
    P = _P
    KT = D // P            # 32 contraction k-tiles
    WCK = 2                # k-tiles per W chunk
    NWC = KT // WCK        # 16 W chunks
    XCK = 4                # k-tiles per x chunk
    NXC = KT // XCK        # 8 x chunks per band
    NB = 4                 # x bands
    BBAND = BPER // NB     # 512 tokens per band
    SBT = BBAND // P       # 4 stripes per band
    NH = OPER // 512       # 2 o-halves (moving dim is 512 max)
    mult = mybir.AluOpType.mult
    add = mybir.AluOpType.add

    nc = bacc.Bacc("TRN2", target_bir_lowering=False, debug=False,
                   num_devices=_NCORES)
    xt_d = nc.dram_tensor("xt", [D, BPER], f32, kind="ExternalInput").ap()
    wt_d = nc.dram_tensor("wt", [D, OPER], f32, kind="ExternalInput").ap()
    b_d = nc.dram_tensor("b", [OPER], f32, kind="ExternalInput").ap()
    o_d = nc.dram_tensor("out", [BPER, OPER], f32, kind="ExternalOutput").ap()

    with tile.TileContext(nc) as tc:
        with (
            tc.tile_pool(name="const", bufs=1) as cpool,
            tc.tile_pool(name="wq", bufs=1) as wqpool,
            tc.tile_pool(name="xq", bufs=2) as xqpool,
            tc.tile_pool(name="wstg", bufs=3) as wstgpool,
            tc.tile_pool(name="xstg", bufs=3) as xstgpool,
            tc.tile_pool(name="mm", bufs=8, space="PSUM") as mmpool,
            tc.tile_pool(name="osb", bufs=3) as opool,
        ):
            bias_bc = cpool.tile([P, OPER], f32)
            nc.sync.dma_start(bias_bc, b_d[None, :].to_broadcast((P, OPER)))

            # All compute tiles are 2D: wq chunk columns are k-major
            # ((kt%WCK)*OPER + o), xq chunk columns ((kt%XCK)*BBAND + b).
            wq = [wqpool.tile([P, WCK * OPER], f16, tag=f"wq{c}",
                              name=f"wq{c}")
                  for c in range(NWC)]

            def quant(dst16, src32):
                # fl32(fl32(100*v) + MAGIC) on DVE (rounds half-to-even
                # onto the integer grid), ACT subtracts MAGIC -> fp16.
                nc.vector.tensor_scalar(src32, src32, 100.0, _MAGIC,
                                        mult, add)
                nc.scalar.activation(dst16, src32,
                                     mybir.ActivationFunctionType.Copy,
                                     bias=-_MAGIC, scale=1.0)

            def load_w_chunk(c):
                st = wstgpool.tile([P, WCK * OPER], f32, tag="wst",
                                   name=f"wst{c}")
                src = wt_d[c * WCK * P:(c + 1) * WCK * P, :].rearrange(
                    "(k p) o -> p k o", p=P)
                nc.sync.dma_start(st.rearrange("p (k o) -> p k o", k=WCK),
                                  src)
                quant(wq[c], st)

            def load_x_chunk(band, c):
                st = xstgpool.tile([P, XCK * BBAND], f32, tag="xst",
                                   name=f"xst{band}_{c}")
                src = xt_d[c * XCK * P:(c + 1) * XCK * P,
                           band * BBAND:(band + 1) * BBAND].rearrange(
                    "(k p) b -> p k b", p=P)
                nc.sync.dma_start(st.rearrange("p (k b) -> p k b", k=XCK),
                                  src)
                t = xqpool.tile([P, XCK * BBAND], f16, tag=f"xq{c}",
                                name=f"xq{band}_{c}")
                quant(t, st)
                return t

            def load_band(band):
                return [load_x_chunk(band, c) for c in range(NXC)]

            def mm(ps, xqt, kt, bt, oh, start, stop):
                xof = (kt % XCK) * BBAND + bt * P
                wof = (kt % WCK) * OPER + oh * 512
                nc.tensor.matmul(
                    ps,
                    xqt[kt // XCK][:, xof:xof + P],
                    wq[kt // WCK][:, wof:wof + 512],
                    start=start, stop=stop)

            def epilogue_bt(band, bt, ps_pair):
                ob = opool.tile([P, OPER], f32, tag="ob",
                                name=f"ob{band}_{bt}")
                for oh in range(NH):
                    nc.vector.scalar_tensor_tensor(
                        ob[:, oh * 512:(oh + 1) * 512], ps_pair[oh], 1e-4,
                        bias_bc[:, oh * 512:(oh + 1) * 512], mult, add)
                row = (band * SBT + bt) * P
                nc.scalar.dma_start(o_d[row:row + P, :], ob)

            # Ramp: per 4-kt group, two W chunks then the matching band-0
            # x chunk, so matmul kt-groups unlock as soon as their data
            # lands; then prefetch band 1.
            xq_tiles = [None] * NB
            xq_tiles[0] = []
            for g in range(NXC):
                load_w_chunk(2 * g)
                load_w_chunk(2 * g + 1)
                xq_tiles[0].append(load_x_chunk(0, g))
            xq_tiles[1] = load_band(1)

            chains = [(bt, oh) for bt in range(SBT) for oh in range(NH)]

            # Band 0: kt-outer across all 8 PSUM banks (DMA-paced ramp).
            ps0 = [mmpool.tile([P, 512], f32, tag="ps", name=f"ps0_{j}")
                   for j in range(len(chains))]
            for kt in range(KT):
                for j, (bt, oh) in enumerate(chains):
                    mm(ps0[j], xq_tiles[0], kt, bt, oh,
                       start=(kt == 0), stop=(kt == KT - 1))
            for bt in range(SBT):
                epilogue_bt(0, bt, ps0[bt * NH:(bt + 1) * NH])
            xq_tiles[2] = load_band(2)

            # Bands 1-3: fully prefetched, chain-sequential with
            # per-stripe epilogues.
            for band in range(1, NB):
                for bt in range(SBT):
                    ps_pair = []
                    for oh in range(NH):
                        ps = mmpool.tile([P, 512], f32, tag="ps",
                                         name=f"ps{band}_{bt}_{oh}")
                        for kt in range(KT):
                            mm(ps, xq_tiles[band], kt, bt, oh,
                               start=(kt == 0), stop=(kt == KT - 1))
                        ps_pair.append(ps)
                    epilogue_bt(band, bt, ps_pair)
                    if band + 2 < NB and bt == 0:
                        xq_tiles[band + 2] = load_band(band + 2)

    nc.compile()
    return nc


def _get_nc(BPER=_BPER, D=_D, OPER=_OPER):
    key = (BPER, D, OPER)
    if key not in _nc_cache:
        _nc_cache[key] = _build(BPER, D, OPER)
    return _nc_cache[key]


def _make_in_maps(x, W, b):
    xt = [np.ascontiguousarray(x[r * _BPER:(r + 1) * _BPER, :].T)
          for r in range(_R)]
    wt = [np.ascontiguousarray(W[c * _OPER:(c + 1) * _OPER, :].T)
          for c in range(_C)]
    bs = [np.ascontiguousarray(b[c * _OPER:(c + 1) * _OPER])
          for c in range(_C)]
    in_maps = []
    for r in range(_R):
        for c in range(_C):
            in_maps.append({"xt": xt[r], "wt": wt[c], "b": bs[c]})
    return in_maps


def _assemble(blocks):
    return np.block([[blocks[r * _C + c] for c in range(_C)]
                     for r in range(_R)])


def _run(x, W, b, trace=False):
    from concourse.bass_utils import run_bass_kernel_spmd

    nc = _get_nc()
    in_maps = _make_in_maps(x, W, b)
    res = run_bass_kernel_spmd(nc, in_maps, core_ids=list(range(_NCORES)),
                               trace=trace)
    out = _assemble([res.results[c]["out"] for c in range(_NCORES)])
    return out, res


def kernel(x=None, W=None, b=None):
    x = np.ascontiguousarray(np.asarray(x, dtype=np.float32))
    W = np.ascontiguousarray(np.asarray(W, dtype=np.float32))
    b = np.ascontiguousarray(np.asarray(b, dtype=np.float32))
    out, _ = _run(x, W, b, trace=False)
    return out
